# revision 2
# baseline (speedup 1.0000x reference)
"""Trainium2 Bass kernel for hierarchical loss.

Math: reference computes
    probs = outputs @ A.T            [B, N]
    w     = W[target]                [B, N]
    loss  = sum_b (1 - probs[b].w[b])
Since probs[b].w[b] = outputs[b] @ (A.T @ W[target_b]) = outputs[b] . M[target_b]
with M = W @ A  ([1000, 1000]), the loss is
    loss = B - sum_b outputs[b] . M[target_b]

M's entries are sums of a few powers of two (W entries are dyadic
rationals, A entries are 0/1), so M is exactly representable in bf16.

Device kernel (per core, data-parallel over batch):
  - stream 128-row tiles of outputs (bf16, padded to 1024 classes)
  - gather the 128 matching M rows by target via indirect DMA
  - for each 128-class chunk k: psum[c1,c2] += sum_b O[b,c1] * G[b,c2]
    (TensorE matmul, PSUM accumulation across all tiles and chunks)
  - the diagonal of the accumulated psum holds sum_b win[b] contributions;
    reduce it to a [128,1] vector, host sums across cores.
"""

import numpy as np
import ml_dtypes

NCORES = 8
B = 32768
C = 1000          # real classes
CP = 1024         # padded classes
P = 128
BPC = B // NCORES  # rows per core (4096)
NTILES = BPC // P  # 32

_NC_CACHE = {}


def _build(repeats=1, gather_mode="indirect"):
    import concourse.bass as bass
    import concourse.tile as tile
    from concourse import bacc, mybir
    from concourse.masks import make_identity

    nc = bacc.Bacc("TRN2", target_bir_lowering=False, debug=False,
                   num_devices=NCORES)
    o_ap = nc.dram_tensor("o", [BPC, CP], mybir.dt.bfloat16,
                          kind="ExternalInput").ap()
    m_ap = nc.dram_tensor("m", [C, CP], mybir.dt.bfloat16,
                          kind="ExternalInput").ap()
    t_ap = nc.dram_tensor("t", [P, NTILES], mybir.dt.int32,
                          kind="ExternalInput").ap()
    r_ap = nc.dram_tensor("r", [P, 1], mybir.dt.float32,
                          kind="ExternalOutput").ap()

    nchunk = CP // P

    with tile.TileContext(nc) as tc:
        with tc.tile_pool(name="io", bufs=4) as io_pool, \
             tc.tile_pool(name="single", bufs=1) as single, \
             tc.tile_pool(name="psum", bufs=1, space="PSUM") as psum_pool:
            t_sb = single.tile([P, NTILES], mybir.dt.int32)
            nc.sync.dma_start(t_sb[:], t_ap[:])

            acc = psum_pool.tile([P, P], mybir.dt.float32)
            n_mm = 0
            total_mm = repeats * NTILES * nchunk
            for _rep in range(repeats):
                for i in range(NTILES):
                    o_t = io_pool.tile([P, CP], mybir.dt.bfloat16, tag="o")
                    nc.sync.dma_start(o_t[:], o_ap[i * P:(i + 1) * P, :])
                    g_t = io_pool.tile([P, CP], mybir.dt.bfloat16, tag="g")
                    nc.gpsimd.indirect_dma_start(
                        out=g_t[:], out_offset=None, in_=m_ap[:],
                        in_offset=bass.IndirectOffsetOnAxis(
                            ap=t_sb[:, i:i + 1], axis=0))
                    for k in range(nchunk):
                        nc.tensor.matmul(
                            acc[:],
                            o_t[:, k * P:(k + 1) * P],
                            g_t[:, k * P:(k + 1) * P],
                            start=(n_mm == 0),
                            stop=(n_mm == total_mm - 1))
                        n_mm += 1

            ident = single.tile([P, P], mybir.dt.float32)
            make_identity(nc, ident[:])
            d_t = single.tile([P, P], mybir.dt.float32)
            r_t = single.tile([P, 1], mybir.dt.float32)
            nc.vector.tensor_tensor(
                out=d_t[:], in0=acc[:], in1=ident[:],
                op=mybir.AluOpType.mult)
            if repeats != 1:
                nc.vector.tensor_scalar_mul(d_t[:], d_t[:], 1.0 / repeats)
            nc.vector.tensor_reduce(
                out=r_t[:], in_=d_t[:], axis=mybir.AxisListType.X,
                op=mybir.AluOpType.add)
            nc.sync.dma_start(r_ap[:], r_t[:])

    nc.compile()
    return nc


def _get_nc(repeats=1, gather_mode="indirect"):
    key = (repeats, gather_mode)
    if key not in _NC_CACHE:
        _NC_CACHE[key] = _build(repeats, gather_mode)
    return _NC_CACHE[key]


def _make_in_maps(outputs, target, M):
    bf16 = ml_dtypes.bfloat16
    O = np.zeros((B, CP), dtype=bf16)
    O[:, :C] = outputs.astype(bf16)
    Mp = np.zeros((C, CP), dtype=bf16)
    Mp[:, :C] = M.astype(bf16)
    # per-core targets laid out [P, NTILES]: t_core[p, i] = target[core*BPC + i*P + p]
    t32 = target.astype(np.int32).reshape(NCORES, NTILES, P).transpose(0, 2, 1)
    t32 = np.ascontiguousarray(t32)
    return [{"o": O[c * BPC:(c + 1) * BPC], "m": Mp, "t": t32[c]}
            for c in range(NCORES)]


def kernel(outputs, target, A, W):
    outputs = np.asarray(outputs, dtype=np.float32)
    target = np.asarray(target)
    A = np.asarray(A, dtype=np.float32)
    W = np.asarray(W, dtype=np.float32)
    assert outputs.shape == (B, C) and target.shape == (B,)

    M = W @ A  # [1000, 1000], exact in f32 (small dyadic rationals)

    from concourse.bass_utils import run_bass_kernel_spmd
    nc = _get_nc()
    in_maps = _make_in_maps(outputs, target, M)
    res = run_bass_kernel_spmd(nc, in_maps, list(range(NCORES)))
    total = sum(float(res.results[c]["r"].sum(dtype=np.float64))
                for c in range(NCORES))
    return np.float32(np.float64(B) - total)
